# revision 3
# baseline (speedup 1.0000x reference)
"""Causal self-attention Trainium2 kernel (B=4, S=2048, D=1024, H=16).

Sharding: 8 cores = 4 batches x 2 head-groups (8 heads each).
Megatron-style: column-parallel QKV, row-parallel output projection;
the 2-way partial-sum reduce + bias happens on host at gather time.

Device-side layout (per core, batch b, head-group g):
  xT   [1024, 2048]  x[b] transposed on host (contraction dim on partitions)
  Q^T = Wq_g.T @ x^T   via matmul(lhsT=Wq chunk, rhs=xT chunk)   [512, 2048]
  K^T  same            -> scores S^T[k,q] = K^T.T @ Q^T  (d=64 contraction)
  V    = x @ Wv_g      via matmul(lhsT=xT chunk, rhs=Wv)          [2048, 512]
  P^T = exp(S^T/8) with causal handled by trimming the streamed q-range per
  k-chunk plus one 128x128 triangle mask multiply on diagonal blocks.
  PV:  lhsT = [V_h | ones*64] so PSUM rows 0:64 = O^T, rows 64:128 = the
  softmax denominator replicated -> partition-aligned normalize at eviction.
  Out-proj: y_partial = O^T.T @ Wo_g rows (no bias; host adds bias + pair-sum).
"""
import numpy as np
import ml_dtypes
from contextlib import ExitStack

import concourse.bass as bass
import concourse.tile as tile
import concourse.mybir as mybir
from concourse.bass_utils import run_bass_kernel_spmd

B, S, D, H = 4, 2048, 1024, 16
HD = 64          # head dim
HPC = 8          # heads per core
DG = HPC * HD    # 512 dims per head-group
P = 128
NQ = 512         # q-tile width
NCH = S // P     # 16 k-chunks
NJ = S // NQ     # 4 q-tiles
DT = mybir.dt.bfloat16
NPDT = ml_dtypes.bfloat16

_CACHE = {}


def split_waits(nc, maxw=1):
    """walrus here accepts at most 1 sync-wait per instruction; split extras onto NOPs."""
    for fn in nc.m.functions:
        for bb in fn.blocks:
            insts = list(bb.instructions)
            new_list = []
            changed = False
            for inst in insts:
                si = inst.sync_info
                waits = list(si.on_wait) if si and si.on_wait else []
                if len(waits) > maxw:
                    changed = True
                    head, keep = waits[:-maxw], waits[-maxw:]
                    for i in range(0, len(head), maxw):
                        nop = mybir.InstNoOp(
                            name=f"{inst.name}_wsplit{i}",
                            sync_info=mybir.SyncInfo(on_wait=head[i:i + maxw], on_update=[]),
                            bass_nofuse=True, engine=inst.engine)
                        nc.register_instruction(nop)
                        new_list.append(nop)
                    inst.sync_info = mybir.SyncInfo(
                        on_wait=keep,
                        on_update=list(si.on_update) if si.on_update else [])
                new_list.append(inst)
            if changed:
                bb.instructions = new_list


def build():
    nc = bass.Bass(trn_type="TRN2", target_bir_lowering=False, debug=False)
    xT = nc.dram_tensor("xT", [D, S], DT, kind="ExternalInput").ap()
    wq = nc.dram_tensor("wq", [D, DG], DT, kind="ExternalInput").ap()
    wk = nc.dram_tensor("wk", [D, DG], DT, kind="ExternalInput").ap()
    wv = nc.dram_tensor("wv", [D, DG], DT, kind="ExternalInput").ap()
    wo = nc.dram_tensor("wo", [DG, D], DT, kind="ExternalInput").ap()
    tri = nc.dram_tensor("tri", [P, P], DT, kind="ExternalInput").ap()
    y = nc.dram_tensor("y", [S, D], mybir.dt.float32, kind="ExternalOutput").ap()

    DCH = D // P  # 8 contraction chunks
    with tile.TileContext(nc) as tc, ExitStack() as ctx:
        const = ctx.enter_context(tc.tile_pool(name="const", bufs=1))
        xw = ctx.enter_context(tc.tile_pool(name="xw", bufs=1))
        acts = ctx.enter_context(tc.tile_pool(name="acts", bufs=1))

        # resident tiles
        xT_sb = xw.tile([P, DCH, S], DT)
        for c in range(DCH):
            nc.sync.dma_start(xT_sb[:, c], xT.rearrange("(c p) s -> c p s", p=P)[c])
        wq_sb = xw.tile([P, DCH, DG], DT)
        wk_sb = xw.tile([P, DCH, DG], DT)
        wv_sb = xw.tile([P, DCH, DG], DT)
        nc.sync.dma_start(wq_sb[:], wq.rearrange("(c p) d -> p c d", p=P))
        nc.sync.dma_start(wk_sb[:], wk.rearrange("(c p) d -> p c d", p=P))
        nc.sync.dma_start(wv_sb[:], wv.rearrange("(c p) d -> p c d", p=P))
        wo_sb = xw.tile([P, DG // P, D], DT)
        nc.sync.dma_start(wo_sb[:], wo.rearrange("(c p) o -> p c o", p=P))
        tri_sb = const.tile([P, P], DT)
        nc.sync.dma_start(tri_sb[:], tri[:])

        qT_sb = acts.tile([P, DG // P, S], DT)   # [2-head block, hp, s]
        kT_sb = acts.tile([P, DG // P, S], DT)
        v_sb = acts.tile([P, NCH, HPC, P], DT)   # [k part, chunk, head, V|ones]
        nc.vector.memset(v_sb[:, :, :, HD:], 1.0)
        oT_sb = acts.tile([P, DG // P, S], DT)

        # ---- phase 1: projections ----
        with tc.tile_pool(name="pp", bufs=2, space="PSUM") as pp:
            for i in range(DG // P):          # 4 d-blocks (2 heads each)
                for jj in range(NJ):          # 4 s-tiles of 512
                    for dst_sb, w_sb in ((qT_sb, wq_sb), (kT_sb, wk_sb)):
                        ps = pp.tile([P, NQ], mybir.dt.float32, tag="pp")
                        for c in range(DCH):
                            nc.tensor.matmul(
                                ps[:], w_sb[:, c, bass.ts(i, P)],
                                xT_sb[:, c, bass.ts(jj, NQ)],
                                start=(c == 0), stop=(c == DCH - 1))
                        nc.vector.tensor_copy(dst_sb[:, i, bass.ts(jj, NQ)], ps[:])
            for m in range(NCH):              # V: 16 s-blocks of 128
                ps = pp.tile([P, DG], mybir.dt.float32, tag="pv")
                for c in range(DCH):
                    nc.tensor.matmul(
                        ps[:], xT_sb[:, c, bass.ts(m, P)], wv_sb[:, c, :],
                        start=(c == 0), stop=(c == DCH - 1))
                nc.vector.tensor_copy(
                    v_sb[:, m, :, 0:HD],
                    ps[:].rearrange("p (h d) -> p h d", d=HD))

        # ---- phase 2: attention (flash, transposed layout) ----
        with tc.tile_pool(name="ap", bufs=2, space="PSUM") as apool, \
             tc.tile_pool(name="pt", bufs=6) as pt, \
             tc.tile_pool(name="rc", bufs=2) as rc:
            for hi in range(HPC // 2):   # head pairs share the 128-part blocks
                for j in range(NJ):
                    pos = [apool.tile([P, NQ], mybir.dt.float32, tag=f"po{s}", name=f"po{s}")
                           for s in range(2)]
                    nch = 4 * j + 4
                    for c in range(nch):
                        qo = max(0, P * c - NQ * j)
                        pTs = []
                        # two K=64 scores matmuls on row groups 0-63 / 64-127:
                        # issued back-to-back so they run concurrently on PE
                        pss = [apool.tile([P, NQ], mybir.dt.float32, tag=f"ps{s}", name=f"ps{s}")
                               for s in range(2)]
                        for s in range(2):
                            hb = s * HD
                            nc.tensor.matmul(
                                pss[s][:, qo:NQ],
                                kT_sb[hb:hb + HD, hi, bass.ts(c, P)],
                                qT_sb[hb:hb + HD, hi, NQ * j + qo:NQ * (j + 1)],
                                start=True, stop=True)
                        for s in range(2):
                            pT = pt.tile([P, NQ], DT, tag=f"pT{s}", name=f"pT{s}")
                            nc.scalar.activation(
                                pT[:, qo:NQ], pss[s][:, qo:NQ],
                                mybir.ActivationFunctionType.Exp, scale=float(HD) ** -0.5)
                            if c >= 4 * j:  # diagonal block: triangle mask
                                nc.vector.tensor_tensor(
                                    pT[:, qo:qo + P], pT[:, qo:qo + P], tri_sb[:],
                                    mybir.AluOpType.mult)
                            pTs.append(pT)
                        for s in range(2):
                            nc.tensor.matmul(
                                pos[s][:, qo:NQ], v_sb[:, c, 2 * hi + s, :],
                                pTs[s][:, qo:NQ],
                                start=(c == 0), stop=(c == nch - 1))
                    for s in range(2):
                        hb = s * HD
                        rcp = rc.tile([P, NQ], mybir.dt.float32, tag=f"rcp{s}", name=f"rcp{s}")
                        nc.vector.reciprocal(rcp[HD:P, :], pos[s][HD:P, :])
                        nc.vector.tensor_tensor(
                            oT_sb[hb:hb + HD, hi, bass.ts(j, NQ)],
                            pos[s][0:HD, :], rcp[HD:P, :], mybir.AluOpType.mult)

        # ---- phase 3: output projection (partial; host adds pair + bias) ----
        with tc.tile_pool(name="yp", bufs=3, space="PSUM") as yp, \
             tc.tile_pool(name="ys", bufs=3) as ys:
            for m in range(NCH):
                for n in range(D // NQ):
                    ps = yp.tile([P, NQ], mybir.dt.float32, tag="y")
                    for c in range(DG // P):
                        nc.tensor.matmul(
                            ps[:], oT_sb[:, c, bass.ts(m, P)],
                            wo_sb[:, c, bass.ts(n, NQ)],
                            start=(c == 0), stop=(c == DG // P - 1))
                    ysb = ys.tile([P, NQ], mybir.dt.float32, tag="ysb")
                    nc.vector.tensor_copy(ysb[:], ps[:])
                    nc.sync.dma_start(y[bass.ts(m, P), bass.ts(n, NQ)], ysb[:])

    split_waits(nc)
    return nc


def kernel(x, Wq, Wk, Wv, Wo, bo):
    x, Wq, Wk, Wv, Wo, bo = (np.asarray(a, np.float32) for a in (x, Wq, Wk, Wv, Wo, bo))
    if "nc" not in _CACHE:
        _CACHE["nc"] = build()
    nc = _CACHE["nc"]

    tri = np.triu(np.ones((P, P), np.float32)).astype(NPDT)  # keep q >= k
    in_maps = []
    for core in range(8):
        b, g = core // 2, core % 2
        sl = slice(g * DG, (g + 1) * DG)
        in_maps.append({
            "xT": np.ascontiguousarray(x[b].T).astype(NPDT),
            "wq": Wq[:, sl].astype(NPDT),
            "wk": Wk[:, sl].astype(NPDT),
            "wv": Wv[:, sl].astype(NPDT),
            "wo": np.ascontiguousarray(Wo[sl, :]).astype(NPDT),
            "tri": tri,
        })
    res = run_bass_kernel_spmd(nc, in_maps, list(range(8)))
    out = np.empty((B, S, D), np.float32)
    for b in range(B):
        out[b] = res.results[2 * b]["y"] + res.results[2 * b + 1]["y"] + bo
    return out
